# revision 3
# baseline (speedup 1.0000x reference)
"""Trainium2 Bass kernel for ContextAM (sigmoid spatial attention + CBAM channel gate).

Reference computation (per batch b):
  Q = wq @ X + bq   [8, N]      (X = x[b] as [64, N], N = 96*96 = 9216)
  K = wk @ X + bk   [8, N]
  V = wv @ X + bv   [64, N]
  att = sigmoid(Q^T K)          [N, N]   -- never materialized to HBM
  out = V @ att^T + X           [64, N]
  scale = sigmoid(mlp(mean_n(out)) + mlp(max_n(out)))   [64]
  y = out * scale[:, None]

Sharding: 8 cores = (batch b = core//2) x (n-half h = core%2). Each core
computes out[:, h*4608:(h+1)*4608] for its batch, flash-style: for each
m-tile of 128 columns of att^T, energy -> sigmoid -> accumulate V-matmul
in PSUM. The CBAM pooling needs full-batch stats, exchanged between the
two cores of a pair with a tiny pairwise AllGather (256 floats).
"""

import numpy as np

import concourse.bacc as bacc
import concourse.mybir as mybir
import concourse.tile as tile
from concourse.bass_utils import run_bass_kernel_spmd

F32 = mybir.dt.float32

B, C, H, W = 4, 64, 96, 96
N = H * W            # 9216
C8 = C // 8          # 8
R = C // 16          # 4
N_CORES = 8
NHALF = N // 2       # 4608 columns of out per core
MT = 128             # m-tile (rows of att^T per step)
N_MT = N // MT       # 72
NS = 1024            # n-columns per pass (PSUM-bank limited)


def build_nc():
    nc = bacc.Bacc("TRN2", target_bir_lowering=False, debug=False,
                   enable_asserts=True, num_devices=N_CORES)

    xb = nc.dram_tensor("xb", [C, N], F32, kind="ExternalInput").ap()
    xq = nc.dram_tensor("xq", [C, NHALF], F32, kind="ExternalInput").ap()
    wqT = nc.dram_tensor("wqT", [C, C8], F32, kind="ExternalInput").ap()
    wkT = nc.dram_tensor("wkT", [C, C8], F32, kind="ExternalInput").ap()
    wvTb = nc.dram_tensor("wvTb", [C + 1, C], F32, kind="ExternalInput").ap()
    bqk = nc.dram_tensor("bqk", [C8, 2], F32, kind="ExternalInput").ap()
    w1T = nc.dram_tensor("w1T", [C, R], F32, kind="ExternalInput").ap()
    w2T = nc.dram_tensor("w2T", [R, C], F32, kind="ExternalInput").ap()

    y = nc.dram_tensor("y", [C, NHALF], F32, kind="ExternalOutput").ap()

    cc_in = nc.dram_tensor("cc_in", [1, 2 * C], F32).ap()
    cc_out = nc.dram_tensor("cc_out", [2, 2 * C], F32).ap()

    with tile.TileContext(nc) as tc:
        with (
            tc.tile_pool(name="const", bufs=1) as cpool,
            tc.tile_pool(name="att", bufs=3) as apool,
            tc.tile_pool(name="pe", bufs=3, space="PSUM") as epool,
            tc.tile_pool(name="po", bufs=1, space="PSUM") as opool,
        ):
            # ---- resident SBUF tensors -------------------------------------
            X = cpool.tile([C + 1, N], F32)        # x[b] plus a ones-row
            XQ = cpool.tile([C, NHALF], F32)       # own n-range slice of x[b]
            Kt = cpool.tile([C8, N], F32)
            Qt = cpool.tile([C8, NHALF], F32)
            VT = cpool.tile([MT, N_MT * C], F32)   # V^T as 72 tiles of [128, 64]
            OUT = cpool.tile([C, NHALF], F32)      # attention out + x

            wq_s = cpool.tile([C, C8], F32)
            wk_s = cpool.tile([C, C8], F32)
            wv_s = cpool.tile([C + 1, C], F32)
            bqk_s = cpool.tile([C8, 2], F32)
            w1_s = cpool.tile([C, R], F32)
            w2_s = cpool.tile([R, C], F32)

            nc.sync.dma_start(X[0:C, :], xb[:])
            nc.vector.memset(X[C:C + 1, :], 1.0)
            nc.sync.dma_start(XQ[:], xq[:])
            nc.sync.dma_start(wq_s[:], wqT[:])
            nc.sync.dma_start(wk_s[:], wkT[:])
            nc.sync.dma_start(wv_s[:], wvTb[:])
            nc.sync.dma_start(bqk_s[:], bqk[:])
            nc.sync.dma_start(w1_s[:], w1T[:])
            nc.sync.dma_start(w2_s[:], w2T[:])

            # ---- Q / K projections -----------------------------------------
            for j in range(NHALF // 512):
                pq = epool.tile([C8, 512], F32, tag="pe")
                nc.tensor.matmul(pq[:], wq_s[:], XQ[:, j * 512:(j + 1) * 512],
                                 start=True, stop=True)
                nc.vector.tensor_scalar_add(Qt[:, j * 512:(j + 1) * 512], pq[:],
                                            bqk_s[:, 0:1])
            for j in range(N // 512):
                pk = epool.tile([C8, 512], F32, tag="pe")
                nc.tensor.matmul(pk[:], wk_s[:], X[0:C, j * 512:(j + 1) * 512],
                                 start=True, stop=True)
                nc.vector.tensor_scalar_add(Kt[:, j * 512:(j + 1) * 512], pk[:],
                                            bqk_s[:, 1:2])

            # ---- V^T tiles --------------------------------------------------
            for t in range(N_MT):
                pv = epool.tile([MT, C], F32, tag="pe")
                nc.tensor.matmul(pv[:], X[:, t * MT:(t + 1) * MT], wv_s[:],
                                 start=True, stop=True)
                nc.vector.tensor_copy(VT[:, t * C:(t + 1) * C], pv[:])

            # ---- main flash loop -------------------------------------------
            spans = [(s, min(s + NS, NHALF)) for s in range(0, NHALF, NS)]
            for (s0, s1) in spans:
                ns = s1 - s0
                po = opool.tile([C, NS], F32, tag="po")
                for t in range(N_MT):
                    pe = epool.tile([MT, NS], F32, tag="pe")
                    for jj in range(ns // 512):
                        nc.tensor.matmul(
                            pe[:, jj * 512:(jj + 1) * 512],
                            Kt[:, t * MT:(t + 1) * MT],
                            Qt[:, s0 + jj * 512:s0 + (jj + 1) * 512],
                            start=True, stop=True)
                    at = apool.tile([MT, NS], F32, tag="att")
                    nc.scalar.activation(at[:, 0:ns], pe[:, 0:ns],
                                         mybir.ActivationFunctionType.Sigmoid)
                    for jj in range(ns // 512):
                        nc.tensor.matmul(
                            po[:, jj * 512:(jj + 1) * 512],
                            VT[:, t * C:(t + 1) * C],
                            at[:, jj * 512:(jj + 1) * 512],
                            start=(t == 0), stop=(t == N_MT - 1))
                nc.vector.tensor_add(OUT[:, s0:s1], po[:, 0:ns], XQ[:, s0:s1])

            # ---- partial pooling stats -> pairwise exchange ----------------
            st = cpool.tile([C, 2], F32)
            nc.vector.reduce_sum(st[:, 0:1], OUT[:], axis=mybir.AxisListType.X)
            nc.vector.reduce_max(st[:, 1:2], OUT[:], axis=mybir.AxisListType.X)
            nc.sync.dma_start(cc_in[0:1, 0:C], st[:, 0:1])
            nc.sync.dma_start(cc_in[0:1, C:2 * C], st[:, 1:2])
            nc.gpsimd.collective_compute(
                "AllGather", mybir.AluOpType.bypass,
                ins=[cc_in.opt()], outs=[cc_out.opt()],
                replica_groups=[[0, 1], [2, 3], [4, 5], [6, 7]])

            sums2 = cpool.tile([C, 2], F32)
            maxs2 = cpool.tile([C, 2], F32)
            nc.sync.dma_start(sums2[:, 0:1], cc_out[0:1, 0:C])
            nc.sync.dma_start(sums2[:, 1:2], cc_out[1:2, 0:C])
            nc.sync.dma_start(maxs2[:, 0:1], cc_out[0:1, C:2 * C])
            nc.sync.dma_start(maxs2[:, 1:2], cc_out[1:2, C:2 * C])

            # ---- CBAM channel gate -----------------------------------------
            avgmx = cpool.tile([C, 2], F32)
            nc.vector.reduce_sum(avgmx[:, 0:1], sums2[:], axis=mybir.AxisListType.X)
            nc.vector.tensor_scalar_mul(avgmx[:, 0:1], avgmx[:, 0:1], 1.0 / N)
            nc.vector.reduce_max(avgmx[:, 1:2], maxs2[:], axis=mybir.AxisListType.X)

            ph = epool.tile([R, 2], F32, tag="pe")
            nc.tensor.matmul(ph[:], w1_s[:], avgmx[:], start=True, stop=True)
            hrelu = cpool.tile([R, 2], F32)
            nc.vector.tensor_scalar_max(hrelu[:], ph[:], 0.0)
            ps = epool.tile([C, 2], F32, tag="pe")
            nc.tensor.matmul(ps[:], w2_s[:], hrelu[:], start=True, stop=True)
            ssum = cpool.tile([C, 1], F32)
            nc.vector.reduce_sum(ssum[:], ps[:], axis=mybir.AxisListType.X)
            scale = cpool.tile([C, 1], F32)
            nc.scalar.activation(scale[:], ssum[:],
                                 mybir.ActivationFunctionType.Sigmoid)

            nc.vector.tensor_scalar_mul(OUT[:], OUT[:], scale[:])
            nc.sync.dma_start(y[:], OUT[:])

    nc.compile()
    return nc


_NC_CACHE = None


def _get_nc():
    global _NC_CACHE
    if _NC_CACHE is None:
        _NC_CACHE = build_nc()
    return _NC_CACHE


def build_in_maps(inputs):
    x = np.ascontiguousarray(np.asarray(inputs["x"], np.float32))
    wq = np.asarray(inputs["wq"], np.float32)
    bq = np.asarray(inputs["bq"], np.float32)
    wk = np.asarray(inputs["wk"], np.float32)
    bk = np.asarray(inputs["bk"], np.float32)
    wv = np.asarray(inputs["wv"], np.float32)
    bv = np.asarray(inputs["bv"], np.float32)
    ca_w1 = np.asarray(inputs["ca_w1"], np.float32)
    ca_w2 = np.asarray(inputs["ca_w2"], np.float32)

    wqT = np.ascontiguousarray(wq.T)
    wkT = np.ascontiguousarray(wk.T)
    wvTb = np.ascontiguousarray(np.concatenate([wv.T, bv[None, :]], axis=0))
    bqk = np.ascontiguousarray(np.stack([bq, bk], axis=1))
    w1T = np.ascontiguousarray(ca_w1.T)
    w2T = np.ascontiguousarray(ca_w2.T)

    xf = x.reshape(B, C, N)
    in_maps = []
    for core in range(N_CORES):
        b, h = core // 2, core % 2
        in_maps.append({
            "xb": np.ascontiguousarray(xf[b]),
            "xq": np.ascontiguousarray(xf[b][:, h * NHALF:(h + 1) * NHALF]),
            "wqT": wqT, "wkT": wkT, "wvTb": wvTb, "bqk": bqk,
            "w1T": w1T, "w2T": w2T,
        })
    return in_maps


def assemble_output(results):
    out = np.empty((B, C, N), np.float32)
    for core in range(N_CORES):
        b, h = core // 2, core % 2
        out[b][:, h * NHALF:(h + 1) * NHALF] = results[core]["y"]
    return out.reshape(B, C, H, W)


def kernel(**inputs):
    nc = _get_nc()
    res = run_bass_kernel_spmd(nc, build_in_maps(inputs), list(range(N_CORES)))
    return assemble_output(res.results)


# revision 4
# speedup vs baseline: 1.0227x; 1.0227x over previous
"""Trainium2 Bass kernel for ContextAM (sigmoid spatial attention + CBAM channel gate).

Reference computation (per batch b):
  Q = wq @ X + bq   [8, N]      (X = x[b] as [64, N], N = 96*96 = 9216)
  K = wk @ X + bk   [8, N]
  V = wv @ X + bv   [64, N]
  att = sigmoid(Q^T K)          [N, N]   -- never materialized to HBM
  out = V @ att^T + X           [64, N]
  scale = sigmoid(mlp(mean_n(out)) + mlp(max_n(out)))   [64]
  y = out * scale[:, None]

Sharding: 8 cores = (batch b = core//2) x (n-half h = core%2). Each core
computes out[:, h*4608:(h+1)*4608] for its batch, flash-style: for each
m-tile of 128 columns of att^T, energy -> sigmoid -> accumulate V-matmul
in PSUM. The CBAM pooling needs full-batch stats, exchanged between the
two cores of a pair with a tiny pairwise AllGather (256 floats).
"""

import numpy as np

import concourse.bacc as bacc
import concourse.mybir as mybir
import concourse.tile as tile
from concourse.bass_utils import run_bass_kernel_spmd

F32 = mybir.dt.float32

B, C, H, W = 4, 64, 96, 96
N = H * W            # 9216
C8 = C // 8          # 8
R = C // 16          # 4
N_CORES = 8
NHALF = N // 2       # 4608 columns of out per core
MT = 128             # m-tile (rows of att^T per step)
N_MT = N // MT       # 72
NS = 1024            # n-columns per pass (PSUM-bank limited)


def build_nc():
    nc = bacc.Bacc("TRN2", target_bir_lowering=False, debug=False,
                   enable_asserts=True, num_devices=N_CORES)

    xb = nc.dram_tensor("xb", [C, N], F32, kind="ExternalInput").ap()
    xq = nc.dram_tensor("xq", [C, NHALF], F32, kind="ExternalInput").ap()
    wqT = nc.dram_tensor("wqT", [C, C8], F32, kind="ExternalInput").ap()
    wkT = nc.dram_tensor("wkT", [C, C8], F32, kind="ExternalInput").ap()
    wvTb = nc.dram_tensor("wvTb", [C + 1, C], F32, kind="ExternalInput").ap()
    bqk = nc.dram_tensor("bqk", [C8, 2], F32, kind="ExternalInput").ap()
    w1T = nc.dram_tensor("w1T", [C, R], F32, kind="ExternalInput").ap()
    w2T = nc.dram_tensor("w2T", [R, C], F32, kind="ExternalInput").ap()

    y = nc.dram_tensor("y", [C, NHALF], F32, kind="ExternalOutput").ap()

    cc_in = nc.dram_tensor("cc_in", [1, 2 * C], F32).ap()
    cc_out = nc.dram_tensor("cc_out", [2, 2 * C], F32).ap()

    with tile.TileContext(nc) as tc:
        with (
            tc.tile_pool(name="const", bufs=1) as cpool,
            tc.tile_pool(name="att", bufs=3) as apool,
            tc.tile_pool(name="pe", bufs=3, space="PSUM") as epool,
            tc.tile_pool(name="po", bufs=1, space="PSUM") as opool,
        ):
            # ---- resident SBUF tensors -------------------------------------
            X = cpool.tile([C + 1, N], F32)        # x[b] plus a ones-row
            XQ = cpool.tile([C, NHALF], F32)       # own n-range slice of x[b]
            Kt = cpool.tile([C8, N], F32)
            Qt = cpool.tile([C8, NHALF], F32)
            VT = cpool.tile([MT, N_MT * C], F32)   # V^T as 72 tiles of [128, 64]
            OUT = cpool.tile([C, NHALF], F32)      # attention out + x

            wq_s = cpool.tile([C, C8], F32)
            wk_s = cpool.tile([C, C8], F32)
            wv_s = cpool.tile([C + 1, C], F32)
            bqk_s = cpool.tile([C8, 2], F32)
            w1_s = cpool.tile([C, R], F32)
            w2_s = cpool.tile([R, C], F32)

            nc.sync.dma_start(X[0:C, :], xb[:])
            nc.vector.memset(X[C:C + 1, :], 1.0)
            nc.sync.dma_start(XQ[:], xq[:])
            nc.sync.dma_start(wq_s[:], wqT[:])
            nc.sync.dma_start(wk_s[:], wkT[:])
            nc.sync.dma_start(wv_s[:], wvTb[:])
            nc.sync.dma_start(bqk_s[:], bqk[:])
            nc.sync.dma_start(w1_s[:], w1T[:])
            nc.sync.dma_start(w2_s[:], w2T[:])

            # ---- Q / K projections -----------------------------------------
            for j in range(NHALF // 512):
                pq = epool.tile([C8, 512], F32, tag="pe")
                nc.tensor.matmul(pq[:], wq_s[:], XQ[:, j * 512:(j + 1) * 512],
                                 start=True, stop=True)
                nc.vector.tensor_scalar_add(Qt[:, j * 512:(j + 1) * 512], pq[:],
                                            bqk_s[:, 0:1])
            for j in range(N // 512):
                pk = epool.tile([C8, 512], F32, tag="pe")
                nc.tensor.matmul(pk[:], wk_s[:], X[0:C, j * 512:(j + 1) * 512],
                                 start=True, stop=True)
                nc.vector.tensor_scalar_add(Kt[:, j * 512:(j + 1) * 512], pk[:],
                                            bqk_s[:, 1:2])

            # ---- V^T tiles --------------------------------------------------
            for t in range(N_MT):
                pv = epool.tile([MT, C], F32, tag="pe")
                nc.tensor.matmul(pv[:], X[:, t * MT:(t + 1) * MT], wv_s[:],
                                 start=True, stop=True)
                nc.vector.tensor_copy(VT[:, t * C:(t + 1) * C], pv[:])

            # ---- main flash loop -------------------------------------------
            # Software-pipelined emission: PE streams energy(t) while ACT
            # runs sigmoid(t-1) and PE then consumes att(t-1). Emitting
            # out(t-1) AFTER energy(t) keeps the in-order PE queue from
            # head-of-line blocking on the sigmoid.
            spans = [(s, min(s + NS, NHALF)) for s in range(0, NHALF, NS)]
            for (s0, s1) in spans:
                ns = s1 - s0
                po = opool.tile([C, NS], F32, tag="po")

                def emit_energy(t, s0=s0, ns=ns):
                    pe = epool.tile([MT, NS], F32, tag="pe")
                    for jj in range(ns // 512):
                        nc.tensor.matmul(
                            pe[:, jj * 512:(jj + 1) * 512],
                            Kt[:, t * MT:(t + 1) * MT],
                            Qt[:, s0 + jj * 512:s0 + (jj + 1) * 512],
                            start=True, stop=True)
                    at = apool.tile([MT, NS], F32, tag="att")
                    nc.scalar.activation(at[:, 0:ns], pe[:, 0:ns],
                                         mybir.ActivationFunctionType.Sigmoid)
                    return at

                def emit_out(t, at, po=po, ns=ns):
                    for jj in range(ns // 512):
                        nc.tensor.matmul(
                            po[:, jj * 512:(jj + 1) * 512],
                            VT[:, t * C:(t + 1) * C],
                            at[:, jj * 512:(jj + 1) * 512],
                            start=(t == 0), stop=(t == N_MT - 1))

                at_prev = emit_energy(0)
                for t in range(1, N_MT):
                    at = emit_energy(t)
                    emit_out(t - 1, at_prev)
                    at_prev = at
                emit_out(N_MT - 1, at_prev)
                nc.vector.tensor_add(OUT[:, s0:s1], po[:, 0:ns], XQ[:, s0:s1])

            # ---- partial pooling stats -> pairwise exchange ----------------
            st = cpool.tile([C, 2], F32)
            nc.vector.reduce_sum(st[:, 0:1], OUT[:], axis=mybir.AxisListType.X)
            nc.vector.reduce_max(st[:, 1:2], OUT[:], axis=mybir.AxisListType.X)
            nc.sync.dma_start(cc_in[0:1, 0:C], st[:, 0:1])
            nc.sync.dma_start(cc_in[0:1, C:2 * C], st[:, 1:2])
            nc.gpsimd.collective_compute(
                "AllGather", mybir.AluOpType.bypass,
                ins=[cc_in.opt()], outs=[cc_out.opt()],
                replica_groups=[[0, 1], [2, 3], [4, 5], [6, 7]])

            sums2 = cpool.tile([C, 2], F32)
            maxs2 = cpool.tile([C, 2], F32)
            nc.sync.dma_start(sums2[:, 0:1], cc_out[0:1, 0:C])
            nc.sync.dma_start(sums2[:, 1:2], cc_out[1:2, 0:C])
            nc.sync.dma_start(maxs2[:, 0:1], cc_out[0:1, C:2 * C])
            nc.sync.dma_start(maxs2[:, 1:2], cc_out[1:2, C:2 * C])

            # ---- CBAM channel gate -----------------------------------------
            avgmx = cpool.tile([C, 2], F32)
            nc.vector.reduce_sum(avgmx[:, 0:1], sums2[:], axis=mybir.AxisListType.X)
            nc.vector.tensor_scalar_mul(avgmx[:, 0:1], avgmx[:, 0:1], 1.0 / N)
            nc.vector.reduce_max(avgmx[:, 1:2], maxs2[:], axis=mybir.AxisListType.X)

            ph = epool.tile([R, 2], F32, tag="pe")
            nc.tensor.matmul(ph[:], w1_s[:], avgmx[:], start=True, stop=True)
            hrelu = cpool.tile([R, 2], F32)
            nc.vector.tensor_scalar_max(hrelu[:], ph[:], 0.0)
            ps = epool.tile([C, 2], F32, tag="pe")
            nc.tensor.matmul(ps[:], w2_s[:], hrelu[:], start=True, stop=True)
            ssum = cpool.tile([C, 1], F32)
            nc.vector.reduce_sum(ssum[:], ps[:], axis=mybir.AxisListType.X)
            scale = cpool.tile([C, 1], F32)
            nc.scalar.activation(scale[:], ssum[:],
                                 mybir.ActivationFunctionType.Sigmoid)

            nc.vector.tensor_scalar_mul(OUT[:], OUT[:], scale[:])
            nc.sync.dma_start(y[:], OUT[:])

    nc.compile()
    return nc


_NC_CACHE = None


def _get_nc():
    global _NC_CACHE
    if _NC_CACHE is None:
        _NC_CACHE = build_nc()
    return _NC_CACHE


def build_in_maps(inputs):
    x = np.ascontiguousarray(np.asarray(inputs["x"], np.float32))
    wq = np.asarray(inputs["wq"], np.float32)
    bq = np.asarray(inputs["bq"], np.float32)
    wk = np.asarray(inputs["wk"], np.float32)
    bk = np.asarray(inputs["bk"], np.float32)
    wv = np.asarray(inputs["wv"], np.float32)
    bv = np.asarray(inputs["bv"], np.float32)
    ca_w1 = np.asarray(inputs["ca_w1"], np.float32)
    ca_w2 = np.asarray(inputs["ca_w2"], np.float32)

    wqT = np.ascontiguousarray(wq.T)
    wkT = np.ascontiguousarray(wk.T)
    wvTb = np.ascontiguousarray(np.concatenate([wv.T, bv[None, :]], axis=0))
    bqk = np.ascontiguousarray(np.stack([bq, bk], axis=1))
    w1T = np.ascontiguousarray(ca_w1.T)
    w2T = np.ascontiguousarray(ca_w2.T)

    xf = x.reshape(B, C, N)
    in_maps = []
    for core in range(N_CORES):
        b, h = core // 2, core % 2
        in_maps.append({
            "xb": np.ascontiguousarray(xf[b]),
            "xq": np.ascontiguousarray(xf[b][:, h * NHALF:(h + 1) * NHALF]),
            "wqT": wqT, "wkT": wkT, "wvTb": wvTb, "bqk": bqk,
            "w1T": w1T, "w2T": w2T,
        })
    return in_maps


def assemble_output(results):
    out = np.empty((B, C, N), np.float32)
    for core in range(N_CORES):
        b, h = core // 2, core % 2
        out[b][:, h * NHALF:(h + 1) * NHALF] = results[core]["y"]
    return out.reshape(B, C, H, W)


def kernel(**inputs):
    nc = _get_nc()
    res = run_bass_kernel_spmd(nc, build_in_maps(inputs), list(range(N_CORES)))
    return assemble_output(res.results)


# revision 7
# speedup vs baseline: 1.9548x; 1.9115x over previous
"""Trainium2 Bass kernel for ContextAM (sigmoid spatial attention + CBAM channel gate).

Reference computation (per batch b):
  Q = wq @ X + bq   [8, N]      (X = x[b] as [64, N], N = 96*96 = 9216)
  K = wk @ X + bk   [8, N]
  V = wv @ X + bv   [64, N]
  att = sigmoid(Q^T K)          [N, N]   -- never materialized to HBM
  out = V @ att^T + X           [64, N]
  scale = sigmoid(mlp(mean_n(out)) + mlp(max_n(out)))   [64]
  y = out * scale[:, None]

Sharding: 8 cores = (batch b = core//2) x (n-half h = core%2). Each core
computes out[:, h*4608:(h+1)*4608] for its batch, flash-style: for each
m-tile of 128 columns of att^T, energy -> sigmoid -> accumulate V-matmul
in PSUM. The CBAM pooling needs full-batch stats, exchanged between the
two cores of a pair with a tiny pairwise AllGather (256 floats).
"""

import numpy as np

import concourse.bacc as bacc
import concourse.mybir as mybir
import concourse.tile as tile
from concourse.bass_utils import run_bass_kernel_spmd

F32 = mybir.dt.float32
BF16 = mybir.dt.bfloat16

B, C, H, W = 4, 64, 96, 96
N = H * W            # 9216
C8 = C // 8          # 8
R = C // 16          # 4
N_CORES = 8
NHALF = N // 2       # 4608 columns of out per core
MT = 128             # m-tile (rows of att^T per step)
N_MT = N // MT       # 72
NS = 1024            # n-columns per pass (PSUM-bank limited)


def build_nc():
    nc = bacc.Bacc("TRN2", target_bir_lowering=False, debug=False,
                   enable_asserts=True, num_devices=N_CORES)

    xb = nc.dram_tensor("xb", [C, N], F32, kind="ExternalInput").ap()
    xq = nc.dram_tensor("xq", [C, NHALF], F32, kind="ExternalInput").ap()
    wqT = nc.dram_tensor("wqT", [C, C8], F32, kind="ExternalInput").ap()
    wkT = nc.dram_tensor("wkT", [C, C8], F32, kind="ExternalInput").ap()
    wvTb = nc.dram_tensor("wvTb", [C + 1, C], F32, kind="ExternalInput").ap()
    bqk = nc.dram_tensor("bqk", [C8, 2], F32, kind="ExternalInput").ap()
    w1T = nc.dram_tensor("w1T", [C, R], F32, kind="ExternalInput").ap()
    w2T = nc.dram_tensor("w2T", [R, C], F32, kind="ExternalInput").ap()

    y = nc.dram_tensor("y", [C, NHALF], F32, kind="ExternalOutput").ap()

    cc_in = nc.dram_tensor("cc_in", [1, 2 * C], F32).ap()
    cc_out = nc.dram_tensor("cc_out", [2, 2 * C], F32).ap()

    with tile.TileContext(nc) as tc:
        with (
            tc.tile_pool(name="const", bufs=1) as cpool,
            tc.tile_pool(name="att", bufs=3) as apool,
            tc.tile_pool(name="pe", bufs=3, space="PSUM") as epool,
            tc.tile_pool(name="po", bufs=1, space="PSUM") as opool,
        ):
            # ---- resident SBUF tensors -------------------------------------
            X = cpool.tile([C + 1, N], F32)        # x[b] plus a ones-row
            XQ = cpool.tile([C, NHALF], F32)       # own n-range slice of x[b]
            Kt = cpool.tile([C8, N], BF16)
            Qt = cpool.tile([C8, NHALF], BF16)
            VT = cpool.tile([MT, N_MT * C], BF16)   # V^T as 72 tiles of [128, 64]
            OUT = cpool.tile([C, NHALF], F32)      # attention out + x

            wq_s = cpool.tile([C, C8], F32)
            wk_s = cpool.tile([C, C8], F32)
            wv_s = cpool.tile([C + 1, C], F32)
            bqk_s = cpool.tile([C8, 2], F32)
            w1_s = cpool.tile([C, R], F32)
            w2_s = cpool.tile([R, C], F32)

            nc.sync.dma_start(X[0:C, :], xb[:])
            nc.vector.memset(X[C:C + 1, :], 1.0)
            nc.sync.dma_start(XQ[:], xq[:])
            nc.sync.dma_start(wq_s[:], wqT[:])
            nc.sync.dma_start(wk_s[:], wkT[:])
            nc.sync.dma_start(wv_s[:], wvTb[:])
            nc.sync.dma_start(bqk_s[:], bqk[:])
            nc.sync.dma_start(w1_s[:], w1T[:])
            nc.sync.dma_start(w2_s[:], w2T[:])

            # ---- Q / K projections -----------------------------------------
            for j in range(NHALF // 512):
                pq = epool.tile([C8, 512], F32, tag="pe")
                nc.tensor.matmul(pq[:], wq_s[:], XQ[:, j * 512:(j + 1) * 512],
                                 start=True, stop=True)
                nc.vector.tensor_scalar_add(Qt[:, j * 512:(j + 1) * 512], pq[:],
                                            bqk_s[:, 0:1])
            for j in range(N // 512):
                pk = epool.tile([C8, 512], F32, tag="pe")
                nc.tensor.matmul(pk[:], wk_s[:], X[0:C, j * 512:(j + 1) * 512],
                                 start=True, stop=True)
                nc.vector.tensor_scalar_add(Kt[:, j * 512:(j + 1) * 512], pk[:],
                                            bqk_s[:, 1:2])

            # ---- V^T tiles --------------------------------------------------
            for t in range(N_MT):
                pv = epool.tile([MT, C], F32, tag="pe")
                nc.tensor.matmul(pv[:], X[:, t * MT:(t + 1) * MT], wv_s[:],
                                 start=True, stop=True)
                nc.vector.tensor_copy(VT[:, t * C:(t + 1) * C], pv[:])

            # ---- main flash loop -------------------------------------------
            # Software-pipelined emission: PE streams energy(t) while ACT
            # runs sigmoid(t-1) and PE then consumes att(t-1). Emitting
            # out(t-1) AFTER energy(t) keeps the in-order PE queue from
            # head-of-line blocking on the sigmoid.
            spans = [(s, min(s + NS, NHALF)) for s in range(0, NHALF, NS)]
            for (s0, s1) in spans:
                ns = s1 - s0
                po = opool.tile([C, NS], F32, tag="po")

                def emit_energy(t, s0=s0, ns=ns):
                    pe = epool.tile([MT, NS], F32, tag="pe")
                    for jj in range(ns // 512):
                        nc.tensor.matmul(
                            pe[:, jj * 512:(jj + 1) * 512],
                            Kt[:, t * MT:(t + 1) * MT],
                            Qt[:, s0 + jj * 512:s0 + (jj + 1) * 512],
                            start=True, stop=True)
                    at = apool.tile([MT, NS], BF16, tag="att")
                    nc.scalar.activation(at[:, 0:ns], pe[:, 0:ns],
                                         mybir.ActivationFunctionType.Sigmoid)
                    return at

                def emit_out(t, at, po=po, ns=ns):
                    for jj in range(ns // 512):
                        nc.tensor.matmul(
                            po[:, jj * 512:(jj + 1) * 512],
                            VT[:, t * C:(t + 1) * C],
                            at[:, jj * 512:(jj + 1) * 512],
                            start=(t == 0), stop=(t == N_MT - 1))

                at_prev = emit_energy(0)
                for t in range(1, N_MT):
                    at = emit_energy(t)
                    emit_out(t - 1, at_prev)
                    at_prev = at
                emit_out(N_MT - 1, at_prev)
                nc.vector.tensor_add(OUT[:, s0:s1], po[:, 0:ns], XQ[:, s0:s1])

            # ---- partial pooling stats -> pairwise exchange ----------------
            st = cpool.tile([C, 2], F32)
            nc.vector.reduce_sum(st[:, 0:1], OUT[:], axis=mybir.AxisListType.X)
            nc.vector.reduce_max(st[:, 1:2], OUT[:], axis=mybir.AxisListType.X)
            nc.sync.dma_start(cc_in[0:1, 0:C], st[:, 0:1])
            nc.sync.dma_start(cc_in[0:1, C:2 * C], st[:, 1:2])
            nc.gpsimd.collective_compute(
                "AllGather", mybir.AluOpType.bypass,
                ins=[cc_in.opt()], outs=[cc_out.opt()],
                replica_groups=[[0, 1], [2, 3], [4, 5], [6, 7]])

            sums2 = cpool.tile([C, 2], F32)
            maxs2 = cpool.tile([C, 2], F32)
            nc.sync.dma_start(sums2[:, 0:1], cc_out[0:1, 0:C])
            nc.sync.dma_start(sums2[:, 1:2], cc_out[1:2, 0:C])
            nc.sync.dma_start(maxs2[:, 0:1], cc_out[0:1, C:2 * C])
            nc.sync.dma_start(maxs2[:, 1:2], cc_out[1:2, C:2 * C])

            # ---- CBAM channel gate -----------------------------------------
            avgmx = cpool.tile([C, 2], F32)
            nc.vector.reduce_sum(avgmx[:, 0:1], sums2[:], axis=mybir.AxisListType.X)
            nc.vector.tensor_scalar_mul(avgmx[:, 0:1], avgmx[:, 0:1], 1.0 / N)
            nc.vector.reduce_max(avgmx[:, 1:2], maxs2[:], axis=mybir.AxisListType.X)

            ph = epool.tile([R, 2], F32, tag="pe")
            nc.tensor.matmul(ph[:], w1_s[:], avgmx[:], start=True, stop=True)
            hrelu = cpool.tile([R, 2], F32)
            nc.vector.tensor_scalar_max(hrelu[:], ph[:], 0.0)
            ps = epool.tile([C, 2], F32, tag="pe")
            nc.tensor.matmul(ps[:], w2_s[:], hrelu[:], start=True, stop=True)
            ssum = cpool.tile([C, 1], F32)
            nc.vector.reduce_sum(ssum[:], ps[:], axis=mybir.AxisListType.X)
            scale = cpool.tile([C, 1], F32)
            nc.scalar.activation(scale[:], ssum[:],
                                 mybir.ActivationFunctionType.Sigmoid)

            nc.vector.tensor_scalar_mul(OUT[:], OUT[:], scale[:])
            nc.sync.dma_start(y[:], OUT[:])

    nc.compile()
    return nc


_NC_CACHE = None


def _get_nc():
    global _NC_CACHE
    if _NC_CACHE is None:
        _NC_CACHE = build_nc()
    return _NC_CACHE


def build_in_maps(inputs):
    x = np.ascontiguousarray(np.asarray(inputs["x"], np.float32))
    wq = np.asarray(inputs["wq"], np.float32)
    bq = np.asarray(inputs["bq"], np.float32)
    wk = np.asarray(inputs["wk"], np.float32)
    bk = np.asarray(inputs["bk"], np.float32)
    wv = np.asarray(inputs["wv"], np.float32)
    bv = np.asarray(inputs["bv"], np.float32)
    ca_w1 = np.asarray(inputs["ca_w1"], np.float32)
    ca_w2 = np.asarray(inputs["ca_w2"], np.float32)

    wqT = np.ascontiguousarray(wq.T)
    wkT = np.ascontiguousarray(wk.T)
    wvTb = np.ascontiguousarray(np.concatenate([wv.T, bv[None, :]], axis=0))
    bqk = np.ascontiguousarray(np.stack([bq, bk], axis=1))
    w1T = np.ascontiguousarray(ca_w1.T)
    w2T = np.ascontiguousarray(ca_w2.T)

    xf = x.reshape(B, C, N)
    in_maps = []
    for core in range(N_CORES):
        b, h = core // 2, core % 2
        in_maps.append({
            "xb": np.ascontiguousarray(xf[b]),
            "xq": np.ascontiguousarray(xf[b][:, h * NHALF:(h + 1) * NHALF]),
            "wqT": wqT, "wkT": wkT, "wvTb": wvTb, "bqk": bqk,
            "w1T": w1T, "w2T": w2T,
        })
    return in_maps


def assemble_output(results):
    out = np.empty((B, C, N), np.float32)
    for core in range(N_CORES):
        b, h = core // 2, core % 2
        out[b][:, h * NHALF:(h + 1) * NHALF] = results[core]["y"]
    return out.reshape(B, C, H, W)


def kernel(**inputs):
    nc = _get_nc()
    res = run_bass_kernel_spmd(nc, build_in_maps(inputs), list(range(N_CORES)))
    return assemble_output(res.results)


# revision 8
# speedup vs baseline: 3.1926x; 1.6332x over previous
"""Trainium2 Bass kernel for ContextAM (sigmoid spatial attention + CBAM channel gate).

Reference computation (per batch b):
  Q = wq @ X + bq   [8, N]      (X = x[b] as [64, N], N = 96*96 = 9216)
  K = wk @ X + bk   [8, N]
  V = wv @ X + bv   [64, N]
  att = sigmoid(Q^T K)          [N, N]   -- never materialized to HBM
  out = V @ att^T + X           [64, N]
  scale = sigmoid(mlp(mean_n(out)) + mlp(max_n(out)))   [64]
  y = out * scale[:, None]

Sharding: 8 cores = (batch b = core//2) x (n-half h = core%2). Each core
computes out[:, h*4608:(h+1)*4608] for its batch, flash-style: for each
m-tile of 128 columns of att^T, energy -> sigmoid -> accumulate V-matmul
in PSUM. The CBAM pooling needs full-batch stats, exchanged between the
two cores of a pair with a tiny pairwise AllGather (256 floats).

PE packing: m-tiles are processed in pairs. The two energy matmuls of a
pair run concurrently in different PE row-groups (K=8 weights at
partition bases 0 and 32), and the two out matmuls run concurrently in
different col-groups (tile_position (0,0)/(0,64)), halving PE wall time.
The out accumulators for the two col-groups live in different PSUM banks
so each accumulation group gets its own start=True bank-clear.
"""

import numpy as np

import concourse.bacc as bacc
import concourse.mybir as mybir
import concourse.tile as tile
from concourse.bass_utils import run_bass_kernel_spmd

F32 = mybir.dt.float32
BF16 = mybir.dt.bfloat16

B, C, H, W = 4, 64, 96, 96
N = H * W            # 9216
C8 = C // 8          # 8
R = C // 16          # 4
N_CORES = 8
NHALF = N // 2       # 4608 columns of out per core
MT = 128             # m-tile (rows of att^T per step)
N_MT = N // MT       # 72
NS = 1024            # n-columns per pass (PSUM-bank limited)


def build_nc():
    nc = bacc.Bacc("TRN2", target_bir_lowering=False, debug=False,
                   enable_asserts=True, num_devices=N_CORES)

    xbb = nc.dram_tensor("xbb", [C + 1, N], BF16, kind="ExternalInput").ap()
    xqb = nc.dram_tensor("xqb", [C, NHALF], BF16, kind="ExternalInput").ap()
    xq = nc.dram_tensor("xq", [C, NHALF], F32, kind="ExternalInput").ap()
    wqT = nc.dram_tensor("wqT", [C, C8], BF16, kind="ExternalInput").ap()
    wkT = nc.dram_tensor("wkT", [C, C8], BF16, kind="ExternalInput").ap()
    wvTb = nc.dram_tensor("wvTb", [C + 1, C], BF16, kind="ExternalInput").ap()
    bqk = nc.dram_tensor("bqk", [C8, 2], F32, kind="ExternalInput").ap()
    w1T = nc.dram_tensor("w1T", [C, R], F32, kind="ExternalInput").ap()
    w2T = nc.dram_tensor("w2T", [R, C], F32, kind="ExternalInput").ap()

    y = nc.dram_tensor("y", [C, NHALF], F32, kind="ExternalOutput").ap()

    cc_in = nc.dram_tensor("cc_in", [1, 2 * C], F32).ap()
    cc_out = nc.dram_tensor("cc_out", [2, 2 * C], F32).ap()

    with tile.TileContext(nc) as tc:
        with (
            tc.tile_pool(name="const", bufs=1) as cpool,
            tc.tile_pool(name="att", bufs=3) as apool,
            tc.tile_pool(name="pe", bufs=2, space="PSUM") as epool,
            tc.tile_pool(name="po", bufs=1, space="PSUM") as opool,
        ):
            # ---- resident SBUF tensors -------------------------------------
            X = cpool.tile([C + 1, N], BF16)       # x[b] plus a ones-row (host)
            XQB = cpool.tile([C, NHALF], BF16)     # own n-range slice, bf16
            XQ = cpool.tile([C, NHALF], F32)       # own n-range slice, f32
            Kt = cpool.tile([40, N], BF16)         # K at partition strips 0-7, 32-39
            Qt = cpool.tile([40, NHALF], BF16)     # Q at partition strips 0-7, 32-39
            VT = cpool.tile([MT, N_MT * C], BF16)  # V^T as 72 tiles of [128, 64]
            OUT = cpool.tile([C, NHALF], F32)      # attention out + x

            wq_s = cpool.tile([C, C8], BF16)
            wk_s = cpool.tile([C, C8], BF16)
            wv_s = cpool.tile([C + 1, C], BF16)
            bqk_s = cpool.tile([C8, 2], F32)
            w1_s = cpool.tile([C, R], F32)
            w2_s = cpool.tile([R, C], F32)

            nc.sync.dma_start(X[:], xbb[:])
            nc.sync.dma_start(XQB[:], xqb[:])
            nc.sync.dma_start(XQ[:], xq[:])
            nc.sync.dma_start(wq_s[:], wqT[:])
            nc.sync.dma_start(wk_s[:], wkT[:])
            nc.sync.dma_start(wv_s[:], wvTb[:])
            nc.sync.dma_start(bqk_s[:], bqk[:])
            nc.sync.dma_start(w1_s[:], w1T[:])
            nc.sync.dma_start(w2_s[:], w2T[:])

            # ---- Q / K projections (strip 0-7), then copy to strip 32-39 ---
            for j in range(NHALF // 512):
                pq = epool.tile([C8, 512], F32, tag="pe")
                nc.tensor.matmul(pq[:], wq_s[:], XQB[:, j * 512:(j + 1) * 512],
                                 start=True, stop=True)
                nc.vector.tensor_scalar_add(Qt[0:C8, j * 512:(j + 1) * 512],
                                            pq[:], bqk_s[:, 0:1])
            for j in range(N // 512):
                pk = epool.tile([C8, 512], F32, tag="pe")
                nc.tensor.matmul(pk[:], wk_s[:], X[0:C, j * 512:(j + 1) * 512],
                                 start=True, stop=True)
                nc.vector.tensor_scalar_add(Kt[0:C8, j * 512:(j + 1) * 512],
                                            pk[:], bqk_s[:, 1:2])
            nc.sync.dma_start(Qt[32:40, :], Qt[0:C8, :])
            nc.sync.dma_start(Kt[32:40, :], Kt[0:C8, :])

            # ---- V^T tiles --------------------------------------------------
            for t in range(N_MT):
                pv = epool.tile([MT, C], F32, tag="pe")
                nc.tensor.matmul(pv[:], X[:, t * MT:(t + 1) * MT], wv_s[:],
                                 start=True, stop=True)
                nc.vector.tensor_copy(VT[:, t * C:(t + 1) * C], pv[:])

            # ---- main flash loop (pair-packed, software-pipelined) ---------
            NPAIR = N_MT // 2
            spans = [(s, min(s + NS, NHALF)) for s in range(0, NHALF, NS)]
            for (s0, s1) in spans:
                ns = s1 - s0
                njj = ns // 512
                po = opool.tile([MT, 2 * NS], F32, tag="po")

                def emit_energy(p, s0=s0, njj=njj):
                    tA, tB = 2 * p, 2 * p + 1
                    ats = []
                    for jj in range(njj):
                        pe = epool.tile([MT, NS], F32, tag="pe")
                        c0 = s0 + jj * 512
                        nc.tensor.matmul(
                            pe[:, 0:512],
                            Kt[0:C8, tA * MT:(tA + 1) * MT],
                            Qt[0:C8, c0:c0 + 512],
                            start=True, stop=True, tile_position=(0, 0))
                        nc.tensor.matmul(
                            pe[:, 512:1024],
                            Kt[32:40, tB * MT:(tB + 1) * MT],
                            Qt[32:40, c0:c0 + 512],
                            start=True, stop=True, tile_position=(32, 0))
                        at = apool.tile([MT, NS], BF16, tag="att")
                        nc.scalar.activation(at[:], pe[:],
                                             mybir.ActivationFunctionType.Sigmoid)
                        ats.append(at)
                    return ats

                def emit_out(p, ats, po=po, njj=njj):
                    tA, tB = 2 * p, 2 * p + 1
                    for jj in range(njj):
                        at = ats[jj]
                        nc.tensor.matmul(
                            po[0:C, jj * 512:(jj + 1) * 512],
                            VT[:, tA * C:(tA + 1) * C],
                            at[:, 0:512],
                            start=(p == 0), stop=(p == NPAIR - 1),
                            tile_position=(0, 0))
                        nc.tensor.matmul(
                            po[C:MT, NS + jj * 512:NS + (jj + 1) * 512],
                            VT[:, tB * C:(tB + 1) * C],
                            at[:, 512:1024],
                            start=(p == 0), stop=(p == NPAIR - 1),
                            tile_position=(0, 64))

                ats_prev = emit_energy(0)
                for p in range(1, NPAIR):
                    ats = emit_energy(p)
                    emit_out(p - 1, ats_prev)
                    ats_prev = ats
                emit_out(NPAIR - 1, ats_prev)

                nc.vector.tensor_add(OUT[:, s0:s1], po[0:C, 0:ns], XQ[:, s0:s1])
                nc.vector.tensor_add(OUT[:, s0:s1], OUT[:, s0:s1],
                                     po[C:MT, NS:NS + ns])

            # ---- partial pooling stats -> pairwise exchange ----------------
            st = cpool.tile([C, 2], F32)
            nc.vector.reduce_sum(st[:, 0:1], OUT[:], axis=mybir.AxisListType.X)
            nc.vector.reduce_max(st[:, 1:2], OUT[:], axis=mybir.AxisListType.X)
            nc.sync.dma_start(cc_in[0:1, 0:C], st[:, 0:1])
            nc.sync.dma_start(cc_in[0:1, C:2 * C], st[:, 1:2])
            nc.gpsimd.collective_compute(
                "AllGather", mybir.AluOpType.bypass,
                ins=[cc_in.opt()], outs=[cc_out.opt()],
                replica_groups=[[0, 1], [2, 3], [4, 5], [6, 7]])

            sums2 = cpool.tile([C, 2], F32)
            maxs2 = cpool.tile([C, 2], F32)
            nc.sync.dma_start(sums2[:, 0:1], cc_out[0:1, 0:C])
            nc.sync.dma_start(sums2[:, 1:2], cc_out[1:2, 0:C])
            nc.sync.dma_start(maxs2[:, 0:1], cc_out[0:1, C:2 * C])
            nc.sync.dma_start(maxs2[:, 1:2], cc_out[1:2, C:2 * C])

            # ---- CBAM channel gate -----------------------------------------
            avgmx = cpool.tile([C, 2], F32)
            nc.vector.reduce_sum(avgmx[:, 0:1], sums2[:], axis=mybir.AxisListType.X)
            nc.vector.tensor_scalar_mul(avgmx[:, 0:1], avgmx[:, 0:1], 1.0 / N)
            nc.vector.reduce_max(avgmx[:, 1:2], maxs2[:], axis=mybir.AxisListType.X)

            ph = epool.tile([R, 2], F32, tag="pe")
            nc.tensor.matmul(ph[:], w1_s[:], avgmx[:], start=True, stop=True)
            hrelu = cpool.tile([R, 2], F32)
            nc.vector.tensor_scalar_max(hrelu[:], ph[:], 0.0)
            ps = epool.tile([C, 2], F32, tag="pe")
            nc.tensor.matmul(ps[:], w2_s[:], hrelu[:], start=True, stop=True)
            ssum = cpool.tile([C, 1], F32)
            nc.vector.reduce_sum(ssum[:], ps[:], axis=mybir.AxisListType.X)
            scale = cpool.tile([C, 1], F32)
            nc.scalar.activation(scale[:], ssum[:],
                                 mybir.ActivationFunctionType.Sigmoid)

            nc.vector.tensor_scalar_mul(OUT[:], OUT[:], scale[:])
            nc.sync.dma_start(y[:], OUT[:])

    nc.compile()
    return nc


_NC_CACHE = None


def _get_nc():
    global _NC_CACHE
    if _NC_CACHE is None:
        _NC_CACHE = build_nc()
    return _NC_CACHE


def build_in_maps(inputs):
    import ml_dtypes
    bf16 = ml_dtypes.bfloat16

    x = np.ascontiguousarray(np.asarray(inputs["x"], np.float32))
    wq = np.asarray(inputs["wq"], np.float32)
    bq = np.asarray(inputs["bq"], np.float32)
    wk = np.asarray(inputs["wk"], np.float32)
    bk = np.asarray(inputs["bk"], np.float32)
    wv = np.asarray(inputs["wv"], np.float32)
    bv = np.asarray(inputs["bv"], np.float32)
    ca_w1 = np.asarray(inputs["ca_w1"], np.float32)
    ca_w2 = np.asarray(inputs["ca_w2"], np.float32)

    wqT = np.ascontiguousarray(wq.T.astype(bf16))
    wkT = np.ascontiguousarray(wk.T.astype(bf16))
    wvTb = np.ascontiguousarray(
        np.concatenate([wv.T, bv[None, :]], axis=0).astype(bf16))
    bqk = np.ascontiguousarray(np.stack([bq, bk], axis=1))
    w1T = np.ascontiguousarray(ca_w1.T)
    w2T = np.ascontiguousarray(ca_w2.T)

    xf = x.reshape(B, C, N)
    ones = np.ones((1, N), np.float32)
    in_maps = []
    for core in range(N_CORES):
        b, h = core // 2, core % 2
        xb1 = np.concatenate([xf[b], ones], axis=0)     # [65, N]
        xqf = np.ascontiguousarray(xf[b][:, h * NHALF:(h + 1) * NHALF])
        in_maps.append({
            "xbb": np.ascontiguousarray(xb1.astype(bf16)),
            "xqb": np.ascontiguousarray(xqf.astype(bf16)),
            "xq": xqf,
            "wqT": wqT, "wkT": wkT, "wvTb": wvTb, "bqk": bqk,
            "w1T": w1T, "w2T": w2T,
        })
    return in_maps


def assemble_output(results):
    out = np.empty((B, C, N), np.float32)
    for core in range(N_CORES):
        b, h = core // 2, core % 2
        out[b][:, h * NHALF:(h + 1) * NHALF] = results[core]["y"]
    return out.reshape(B, C, H, W)


def kernel(**inputs):
    nc = _get_nc()
    res = run_bass_kernel_spmd(nc, build_in_maps(inputs), list(range(N_CORES)))
    return assemble_output(res.results)
